# revision 9
# baseline (speedup 1.0000x reference)
"""GCN (2-layer GCNConv + global mean pool + linear head) on 8 Trainium2 cores.

Strategy (follows the sharding hint):
  - Nodes sharded contiguously: core k owns nodes [6272k, 6272k+6272).
  - Edges sharded by destination; per dst-block (112 nodes) edge lists are
    built host-side (index-only preprocessing) and padded to uniform size.
  - GCNConv: out = dis * (sum_{e:dst=v} tab[src_e]) + sf with
    tab = (h @ W) * dis (fp16 table), sf = rec*(h@W) + b (self-loop+bias),
    dis = 1/sqrt(deg+1), rec = dis^2.
  - Neighbor rows are fetched with gpsimd.dma_gather (two <=32768-row table
    halves, int16 indices, 8192 idx/op covering 8 dst blocks), summed per
    dst block with a one-hot matmul on the PE (segment-sum).
  - Per-layer tables are shard-computed then AllGather'd (halo exchange);
    layer-2 table build is fused into the layer-1 scatter loop.
  - Pooling: one-hot (node->graph) matmul -> sums, AllReduce, mean (host
    precomputed 1/count) + linear head on-device.
All numeric compute runs on-device; the host only partitions/reorders
indices and pre-replicates per-node/per-graph normalization constants.
"""
import sys
import types

sys.path.insert(0, "/opt/trn_rl_repo")


def _install_ntff_hook():
    if "antenv.axon_hooks" in sys.modules:
        return
    mod = types.ModuleType("antenv.axon_hooks")
    mod._hook = None
    mod.set_axon_ntff_profile_hook = lambda h: setattr(mod, "_hook", h)
    mod.get_axon_ntff_profile_hook = lambda: mod._hook
    sys.modules["antenv.axon_hooks"] = mod
    sys.path.insert(0, "/root/.axon_site")
    try:
        from trn_agent_boot.trn_boot import _ntff_profile_via_ctypes
        mod.set_axon_ntff_profile_hook(
            _ntff_profile_via_ctypes("/opt/axon/libaxon_pjrt.so"))
    except Exception:
        pass


_install_ntff_hook()

import numpy as np
import concourse.bass as bass
import concourse.bacc as bacc
import concourse.mybir as mybir
import concourse.tile as tile
from concourse import bass_utils
from concourse.masks import make_identity

P = 128
N = 50000
E = 800000
H = 128
C = 10
G = 512
NCORES = 8
BS = 112                 # nodes per dst block
BPC = 56                 # dst blocks per core
NPC = BPC * BS           # nodes per core (6272)
NPAD = NCORES * NPC      # padded node count (50176)
NHALF = NPAD // 2        # table half rows (25088)
CH = 8                   # chunks per half per block (1024 idx)
CPB = 2 * CH             # chunks per block
FW = CH * P // 16        # idx columns per half per block (64)
MERGE = 1                # dst blocks per dma_gather
GRP = BPC // MERGE       # gather groups per core (7)
FWG = MERGE * FW         # idx columns per half per group
NQ = 4

_cache = {}


def _wrap_idx(idx_flat):
    """dma_gather idx layout: position i -> [i%16, i//16], replicated x8."""
    n = idx_flat.shape[0]
    arr = np.ascontiguousarray(idx_flat.reshape(n // 16, 16).T).astype(np.int16)
    return np.tile(arr, (8, 1))


def _prep(x, edge_index, batch, W1, b1, W2, b2, Wl, bl):
    src = np.asarray(edge_index[0], np.int64)
    dst = np.asarray(edge_index[1], np.int64)
    batch = np.asarray(batch, np.int64)
    x = np.asarray(x, np.float32)

    deg = np.bincount(dst, minlength=N).astype(np.float32) + 1.0
    deg_pad = np.ones(NPAD, np.float32)
    deg_pad[:N] = deg
    dis_pad = 1.0 / np.sqrt(deg_pad)

    core_of = dst // NPC
    blk_of = (dst % NPC) // BS

    idxlo = np.zeros((NCORES, P, BPC * FW), np.int16)
    idxhi = np.zeros((NCORES, P, BPC * FW), np.int16)
    dstrel = np.full((NCORES, P, BPC * CPB), -1.0, np.float16)
    for k in range(NCORES):
        m = core_of == k
        sk, dk, bk = src[m], dst[m], blk_of[m]
        order = np.argsort(bk, kind="stable")
        sk, dk, bk = sk[order], dk[order], bk[order]
        bounds = np.searchsorted(bk, np.arange(BPC + 1))
        for b in range(BPC):
            s = sk[bounds[b]:bounds[b + 1]]
            rel = (dk[bounds[b]:bounds[b + 1]] - (k * NPC + b * BS)).astype(np.int64)
            lo = s < NHALF
            slo, rlo = s[lo], rel[lo]
            shi, rhi = s[~lo] - NHALF, rel[~lo]
            assert len(slo) <= CH * P and len(shi) <= CH * P, (len(slo), len(shi))
            lo_full = np.zeros(CH * P, np.int64)
            lo_full[:len(slo)] = slo
            hi_full = np.zeros(CH * P, np.int64)
            hi_full[:len(shi)] = shi
            idxlo[k, :, b * FW:(b + 1) * FW] = _wrap_idx(lo_full)
            idxhi[k, :, b * FW:(b + 1) * FW] = _wrap_idx(hi_full)
            for (rels, coff) in ((rlo, 0), (rhi, CH)):
                nr = len(rels)
                ch = np.arange(nr) // P + coff
                pp = np.arange(nr) % P
                dstrel[k, pp, b * CPB + ch] = rels

    # xT tiles per core: [BPC, 128 feat, BS nodes] fp16 (host layout transform)
    x_pad = np.zeros((NPAD, P), np.float32)
    x_pad[:N] = x
    xt = np.ascontiguousarray(
        x_pad.reshape(NCORES, BPC, BS, P).transpose(0, 1, 3, 2)).astype(np.float16)

    # dis replicated along features: [core][BS, BPC*H]
    dismat = np.ones((NCORES, P, BPC * H), np.float16)
    batf = np.full((NCORES, P, BPC), -1.0, np.float16)
    bat_pad = np.full(NPAD, -1.0, np.float32)
    bat_pad[:N] = batch.astype(np.float32)
    for k in range(NCORES):
        dblk = dis_pad[k * NPC:(k + 1) * NPC].reshape(BPC, BS).T  # [BS, BPC]
        dismat[k, :BS] = np.repeat(dblk[:, :, None], H, axis=2).reshape(
            BS, BPC * H).astype(np.float16)
        batf[k, :BS] = bat_pad[k * NPC:(k + 1) * NPC].reshape(BPC, BS).T

    iota_rep = np.tile(np.arange(BS, dtype=np.float16), (P, CPB))
    iota512 = np.tile(np.arange(G, dtype=np.float16), (P, 1))

    cnt = np.bincount(batch, minlength=G).astype(np.float32)
    recg = np.tile((1.0 / np.clip(cnt, 1.0, None))[None, :], (P, 1))

    common = {
        "W1f": np.asarray(W1, np.float16),
        "W2f": np.asarray(W2, np.float16),
        "Wlf": np.asarray(Wl, np.float32),
        "b1r": np.tile(np.asarray(b1, np.float16), (P, 1)),
        "b2r": np.tile(np.asarray(b2, np.float16), (P, 1)),
        "blc": np.asarray(bl, np.float32).reshape(C, 1),
        "iota_rep": np.ascontiguousarray(iota_rep),
        "iota512": np.ascontiguousarray(iota512),
        "recg": np.ascontiguousarray(recg, np.float32),
    }
    in_maps = []
    for k in range(NCORES):
        m = dict(common)
        m["xt"] = xt[k]
        m["idxlo"] = idxlo[k]
        m["idxhi"] = idxhi[k]
        m["dstrel"] = dstrel[k]
        m["dismat"] = dismat[k]
        m["batf"] = batf[k]
        in_maps.append(m)
    return in_maps


def _build():
    RG = [list(range(NCORES))]
    f32, f16 = mybir.dt.float32, mybir.dt.float16
    MUL, ADD, EQ = (mybir.AluOpType.mult, mybir.AluOpType.add,
                    mybir.AluOpType.is_equal)

    nc = bacc.Bacc("TRN2", target_bir_lowering=False, debug=False,
                   num_devices=NCORES, num_swdge_queues=NQ)

    def inp(name, shape, dt):
        return nc.dram_tensor(name, shape, dt, kind="ExternalInput").ap()

    xt = inp("xt", (BPC, P, BS), f16)
    idxlo = inp("idxlo", (P, BPC * FW), mybir.dt.int16)
    idxhi = inp("idxhi", (P, BPC * FW), mybir.dt.int16)
    dstrel = inp("dstrel", (P, BPC * CPB), f16)
    dismat = inp("dismat", (P, BPC * H), f16)
    batf = inp("batf", (P, BPC), f16)
    W1f = inp("W1f", (P, H), f16)
    W2f = inp("W2f", (P, H), f16)
    Wlf = inp("Wlf", (H, C), f32)
    b1r = inp("b1r", (P, H), f16)
    b2r = inp("b2r", (P, H), f16)
    blc = inp("blc", (C, 1), f32)
    iota_rep = inp("iota_rep", (P, CPB * BS), f16)
    iota512 = inp("iota512", (P, G), f16)
    recg = inp("recg", (P, G), f32)
    out = nc.dram_tensor("out", (G, C), f32, kind="ExternalOutput").ap()

    with tile.TileContext(nc) as tc:
        with tc.tile_pool(name="const", bufs=1) as cpool, \
             tc.tile_pool(name="dram", bufs=1, space="DRAM") as dpool, \
             tc.tile_pool(name="wtile", bufs=3) as wpool, \
             tc.tile_pool(name="gath", bufs=2) as gpool, \
             tc.tile_pool(name="oh", bufs=3) as ohpool, \
             tc.tile_pool(name="ep", bufs=3) as eppool, \
             tc.tile_pool(name="persist", bufs=1) as ppool:

            tab_shard = [dpool.tile([NPC, H], f16, tag=f"tsh{l}", name=f"tsh{l}")
                         for l in (1, 2)]
            tab_full = [dpool.tile([NPAD, H], f16, addr_space="Shared",
                                   tag=f"tfl{l}", name=f"tfl{l}") for l in (1, 2)]
            ar_in = dpool.tile([P, G], f32, tag="ar_in", name="ar_in")
            ar_out = dpool.tile([P, G], f32, addr_space="Shared",
                                tag="ar_out", name="ar_out")

            # ---- constants ---------------------------------------------
            W1_sb = cpool.tile([P, H], f16)
            nc.sync.dma_start(out=W1_sb[:], in_=W1f[:, :])
            dis_sb = cpool.tile([P, BPC * H], f16)
            nc.sync.dma_start(out=dis_sb[:], in_=dismat[:, :])
            b1_sb = cpool.tile([P, H], f16)
            nc.sync.dma_start(out=b1_sb[:], in_=b1r[:, :])
            W2_sb = cpool.tile([P, H], f16)
            nc.sync.dma_start(out=W2_sb[:], in_=W2f[:, :])
            b2_sb = cpool.tile([P, H], f16)
            nc.sync.dma_start(out=b2_sb[:], in_=b2r[:, :])
            idxlo_sb = cpool.tile([P, BPC * FW], mybir.dt.int16)
            nc.sync.dma_start(out=idxlo_sb[:], in_=idxlo[:, :])
            idxhi_sb = cpool.tile([P, BPC * FW], mybir.dt.int16)
            nc.sync.dma_start(out=idxhi_sb[:], in_=idxhi[:, :])
            dst_sb = cpool.tile([P, BPC * CPB], f16)
            nc.sync.dma_start(out=dst_sb[:], in_=dstrel[:, :])
            iota_sb = cpool.tile([P, CPB * BS], f16)
            nc.sync.dma_start(out=iota_sb[:], in_=iota_rep[:, :])
            iota512_sb = cpool.tile([P, G], f16)
            nc.sync.dma_start(out=iota512_sb[:], in_=iota512[:, :])
            bat_sb = cpool.tile([P, BPC], f16)
            nc.sync.dma_start(out=bat_sb[:], in_=batf[:, :])
            Wl_sb = cpool.tile([H, C], f32)
            nc.sync.dma_start(out=Wl_sb[:], in_=Wlf[:, :])
            bl_sb = cpool.tile([C, 1], f32)
            nc.sync.dma_start(out=bl_sb[:], in_=blc[:, :])
            recg_sb = cpool.tile([P, G], f32)
            nc.sync.dma_start(out=recg_sb[:], in_=recg[:, :])
            identh = cpool.tile([P, P], f16)
            make_identity(nc, identh[:])
            identf = cpool.tile([P, P], f32)
            make_identity(nc, identf[:])

            sf_sb = [ppool.tile([P, BPC * H], f16, tag=f"sf{l}", name=f"sf{l}")
                     for l in (1, 2)]

            def dis_blk(b):
                return dis_sb[:BS, b * H:(b + 1) * H]

            with tc.tile_pool(name="psAD", bufs=2, space="PSUM") as pspool, \
                 tc.tile_pool(name="psPool", bufs=1, space="PSUM") as plpool:
                ps_pool = plpool.tile([P, G], f32, tag="pool")

                # ---- layer-1 table + self rows -------------------------
                for t in range(BPC):
                    xf = wpool.tile([P, BS], f16, tag="xf")
                    nc.sync.dma_start(out=xf[:], in_=xt[t, :, :])
                    psm = pspool.tile([BS, H], f32, tag="psm")
                    nc.tensor.matmul(out=psm[:], lhsT=xf[:], rhs=W1_sb[:],
                                     start=True, stop=True)
                    tt = wpool.tile([BS, H], f16, tag="tt")
                    nc.vector.tensor_tensor(out=tt[:], in0=psm[:],
                                            in1=dis_blk(t), op=MUL)
                    nc.sync.dma_start(out=tab_shard[0][t * BS:(t + 1) * BS, :],
                                      in_=tt[:])
                    tsf = wpool.tile([BS, H], f16, tag="tsf")
                    nc.vector.tensor_tensor(out=tsf[:], in0=tt[:],
                                            in1=dis_blk(t), op=MUL)
                    nc.vector.tensor_tensor(
                        out=sf_sb[0][:BS, t * H:(t + 1) * H], in0=tsf[:],
                        in1=b1_sb[:BS, :], op=ADD)
                nc.gpsimd.collective_compute(
                    "AllGather", mybir.AluOpType.bypass, replica_groups=RG,
                    ins=[tab_shard[0][:, :]], outs=[tab_full[0][:, :]])

                # ---- scatter layers ------------------------------------
                def gather_group(l, g):
                    gt = gpool.tile([P, MERGE * CPB * P], f16, tag="gt")
                    gt3 = gt[:].rearrange("p (c e) -> p c e", e=P)
                    nc.gpsimd.dma_gather(
                        out_ap=gt3[:, 0:MERGE * CH, :],
                        in_ap=tab_full[l][0:NHALF, :],
                        idxs_ap=idxlo_sb[:, g * FWG:(g + 1) * FWG],
                        num_idxs=MERGE * CH * P, num_idxs_reg=MERGE * CH * P,
                        elem_size=H, queue_num=(2 * g) % NQ)
                    nc.gpsimd.dma_gather(
                        out_ap=gt3[:, MERGE * CH:MERGE * CPB, :],
                        in_ap=tab_full[l][NHALF:NPAD, :],
                        idxs_ap=idxhi_sb[:, g * FWG:(g + 1) * FWG],
                        num_idxs=MERGE * CH * P, num_idxs_reg=MERGE * CH * P,
                        elem_size=H, queue_num=(2 * g + 1) % NQ)
                    return gt

                def block_agg(gt, j, b):
                    oh = ohpool.tile([P, CPB * BS], f16, tag="oh")
                    nc.vector.tensor_tensor(
                        out=oh[:].rearrange("p (c e) -> p c e", e=BS),
                        in0=dst_sb[:, b * CPB:(b + 1) * CPB][:, :, None]
                            .to_broadcast([P, CPB, BS]),
                        in1=iota_sb[:].rearrange("p (c e) -> p c e", e=BS),
                        op=EQ)
                    agg = pspool.tile([BS, H], f32, tag="agg")
                    for c in range(CPB):
                        col = (j * CH + c) if c < CH else \
                            (MERGE * CH + j * CH + (c - CH))
                        nc.tensor.matmul(out=agg[:],
                                         lhsT=oh[:, c * BS:(c + 1) * BS],
                                         rhs=gt[:, col * P:(col + 1) * P],
                                         start=(c == 0), stop=(c == CPB - 1))
                    return agg

                def epilogue(agg, l, b, dt):
                    t1 = eppool.tile([BS, H], f16, tag="t1")
                    nc.vector.tensor_tensor(out=t1[:], in0=agg[:],
                                            in1=dis_blk(b), op=MUL)
                    u = eppool.tile([BS, H], f16, tag="u")
                    nc.vector.tensor_tensor(
                        out=u[:], in0=t1[:],
                        in1=sf_sb[l][:BS, b * H:(b + 1) * H], op=ADD)
                    ht = eppool.tile([BS, H], dt, tag="ht")
                    nc.scalar.activation(
                        out=ht[:], in_=u[:],
                        func=mybir.ActivationFunctionType.Relu)
                    return ht

                # layer 1 scatter, fused with layer-2 table build
                for g in range(GRP):
                    gt = gather_group(0, g)
                    for j in range(MERGE):
                        b = g * MERGE + j
                        agg = block_agg(gt, j, b)
                        h1t = epilogue(agg, 0, b, f32)
                        pst = pspool.tile([P, BS], f32, tag="pst")
                        nc.tensor.transpose(out=pst[:], in_=h1t[:, :],
                                            identity=identf[:BS, :BS])
                        hT = eppool.tile([P, BS], f16, tag="hT")
                        nc.scalar.activation(
                            out=hT[:], in_=pst[:],
                            func=mybir.ActivationFunctionType.Copy)
                        psm2 = pspool.tile([BS, H], f32, tag="psm")
                        nc.tensor.matmul(out=psm2[:], lhsT=hT[:], rhs=W2_sb[:],
                                         start=True, stop=True)
                        tt2 = eppool.tile([BS, H], f16, tag="tt2")
                        nc.vector.tensor_tensor(out=tt2[:], in0=psm2[:],
                                                in1=dis_blk(b), op=MUL)
                        nc.sync.dma_start(
                            out=tab_shard[1][b * BS:(b + 1) * BS, :],
                            in_=tt2[:])
                        ts2 = eppool.tile([BS, H], f16, tag="ts2")
                        nc.vector.tensor_tensor(out=ts2[:], in0=tt2[:],
                                                in1=dis_blk(b), op=MUL)
                        nc.vector.tensor_tensor(
                            out=sf_sb[1][:BS, b * H:(b + 1) * H], in0=ts2[:],
                            in1=b2_sb[:BS, :], op=ADD)
                nc.gpsimd.collective_compute(
                    "AllGather", mybir.AluOpType.bypass, replica_groups=RG,
                    ins=[tab_shard[1][:, :]], outs=[tab_full[1][:, :]])

                # layer 2 scatter + pooling
                for g in range(GRP):
                    gt = gather_group(1, g)
                    for j in range(MERGE):
                        b = g * MERGE + j
                        agg = block_agg(gt, j, b)
                        h2t = epilogue(agg, 1, b, f16)
                        oh5 = ohpool.tile([P, G], f16, tag="oh5")
                        nc.vector.tensor_tensor(
                            out=oh5[:],
                            in0=bat_sb[:, b:b + 1].to_broadcast([P, G]),
                            in1=iota512_sb[:], op=EQ)
                        nc.tensor.matmul(out=ps_pool[:], lhsT=h2t[:, :],
                                         rhs=oh5[:BS, :],
                                         start=(b == 0), stop=(b == BPC - 1))

                # ---- pooling tail --------------------------------------
                sums_sb = ppool.tile([P, G], f32, tag="sums")
                nc.vector.tensor_copy(out=sums_sb[:], in_=ps_pool[:])
                nc.sync.dma_start(out=ar_in[:, :], in_=sums_sb[:])
            nc.gpsimd.collective_compute(
                "AllReduce", mybir.AluOpType.add, replica_groups=RG,
                ins=[ar_in[:, :]], outs=[ar_out[:, :]])
            psE = tc.tile_pool(name="psE", bufs=1, space="PSUM")
            pspool = psE.__enter__()
            sums2 = ppool.tile([P, G], f32, tag="sums2")
            nc.sync.dma_start(out=sums2[:], in_=ar_out[:, :])
            means = ppool.tile([P, G], f32, tag="means")
            nc.vector.tensor_tensor(out=means[:], in0=sums2[:], in1=recg_sb[:],
                                    op=MUL)
            ps_out = pspool.tile([C, G], f32, tag="out")
            nc.tensor.matmul(out=ps_out[:], lhsT=Wl_sb[:], rhs=means[:],
                             start=True, stop=True)
            outT = ppool.tile([C, G], f32, tag="outT")
            nc.scalar.activation(out=outT[:], in_=ps_out[:],
                                 func=mybir.ActivationFunctionType.Identity,
                                 bias=bl_sb[:, 0:1])
            for g in range(G // P):
                ps_tr = pspool.tile([P, C], f32, tag="tr")
                nc.tensor.transpose(out=ps_tr[:],
                                    in_=outT[:, g * P:(g + 1) * P],
                                    identity=identf[:C, :C])
                ot = eppool.tile([P, C], f32, tag="ot")
                nc.vector.tensor_copy(out=ot[:], in_=ps_tr[:])
                nc.sync.dma_start(out=out[g * P:(g + 1) * P, :], in_=ot[:])
            psE.__exit__(None, None, None)

    nc.compile()
    return nc


def kernel(x, edge_index, batch, W1, b1, W2, b2, Wl, bl, _trace=False):
    in_maps = _prep(x, edge_index, batch, W1, b1, W2, b2, Wl, bl)
    if "nc" not in _cache:
        _cache["nc"] = _build()
    nc = _cache["nc"]
    res = bass_utils.run_bass_kernel_spmd(
        nc, in_maps, core_ids=list(range(NCORES)), trace=_trace)
    kernel.last_result = res
    return res.results[0]["out"].astype(np.float32)


# revision 23
# speedup vs baseline: 1.4576x; 1.4576x over previous
"""GCN (2-layer GCNConv + global mean pool + linear head) on 8 Trainium2 cores.

Strategy (follows the sharding hint):
  - Nodes sharded contiguously: core k owns nodes [6272k, 6272k+6272).
  - Edges sharded by destination; per dst-block (112 nodes) edge lists are
    built host-side (index-only preprocessing) and padded to uniform size.
  - GCNConv: out = dis * (sum_{e:dst=v} tab[src_e]) + sf with
    tab = (h @ W) * dis (fp16 table), sf = rec*(h@W) + b (self-loop+bias),
    dis = 1/sqrt(deg+1), rec = dis^2.
  - Neighbor rows are fetched with gpsimd.dma_gather (two <=32768-row table
    halves, int16 indices, 1024 idx/op), summed per dst block with a
    one-hot matmul on the PE (segment-sum). The block loop is software
    pipelined: gathers lead by 3 blocks, one-hot builds by 1.
  - Per-layer tables are shard-computed then AllGather'd (halo exchange);
    layer-2 table build is fused into the layer-1 scatter loop.
  - Pooling: one-hot (node->graph) matmul -> sums, AllReduce, mean (host
    precomputed 1/count) + linear head on-device.
All numeric compute runs on-device; the host only partitions/reorders
indices and pre-replicates per-node/per-graph normalization constants.
"""
import sys
import types

sys.path.insert(0, "/opt/trn_rl_repo")


def _install_ntff_hook():
    if "antenv.axon_hooks" in sys.modules:
        return
    mod = types.ModuleType("antenv.axon_hooks")
    mod._hook = None
    mod.set_axon_ntff_profile_hook = lambda h: setattr(mod, "_hook", h)
    mod.get_axon_ntff_profile_hook = lambda: mod._hook
    sys.modules["antenv.axon_hooks"] = mod
    sys.path.insert(0, "/root/.axon_site")
    try:
        from trn_agent_boot.trn_boot import _ntff_profile_via_ctypes
        mod.set_axon_ntff_profile_hook(
            _ntff_profile_via_ctypes("/opt/axon/libaxon_pjrt.so"))
    except Exception:
        pass


_install_ntff_hook()

import numpy as np
import concourse.bass as bass
import concourse.bacc as bacc
import concourse.mybir as mybir
import concourse.tile as tile
from concourse import bass_utils
from concourse.masks import make_identity

P = 128
N = 50000
E = 800000
H = 128
C = 10
G = 512
NCORES = 8
BS = 112                 # nodes per dst block
BPC = 56                 # dst blocks per core
NPC = BPC * BS           # nodes per core (6272)
NPAD = NCORES * NPC      # padded node count (50176)
NHALF = NPAD // 2        # table half rows (25088)
CH = 8                   # chunks per half per block (1024 idx = one gather)
CPB = 2 * CH             # chunks per block
SPB = CPB * P            # slots per block (2048)
FW = CH * P // 16        # idx columns per half per block (64)
NQ = 4
LEAD_G = 3               # gather lookahead (blocks)
LEAD_O = 1               # one-hot lookahead (blocks)

_cache = {}


def _wrap_idx(idx_flat):
    """dma_gather idx layout: position i -> [i%16, i//16], replicated x8."""
    n = idx_flat.shape[0]
    arr = np.ascontiguousarray(idx_flat.reshape(n // 16, 16).T).astype(np.int16)
    return np.tile(arr, (8, 1))


def _prep(x, edge_index, batch, W1, b1, W2, b2, Wl, bl):
    src = np.asarray(edge_index[0], np.int64)
    dst = np.asarray(edge_index[1], np.int64)
    batch = np.asarray(batch, np.int64)
    x = np.asarray(x, np.float32)

    deg = np.bincount(dst, minlength=N).astype(np.float32) + 1.0
    deg_pad = np.ones(NPAD, np.float32)
    deg_pad[:N] = deg
    dis_pad = 1.0 / np.sqrt(deg_pad)

    core_of = dst // NPC
    blk_of = (dst % NPC) // BS

    idxlo = np.zeros((NCORES, P, BPC * FW), np.int16)
    idxhi = np.zeros((NCORES, P, BPC * FW), np.int16)
    dstrel = np.full((NCORES, P, BPC * CPB), -1.0, np.float16)
    for k in range(NCORES):
        m = core_of == k
        sk, dk, bk = src[m], dst[m], blk_of[m]
        order = np.argsort(bk, kind="stable")
        sk, dk, bk = sk[order], dk[order], bk[order]
        bounds = np.searchsorted(bk, np.arange(BPC + 1))
        for b in range(BPC):
            s = sk[bounds[b]:bounds[b + 1]]
            rel = (dk[bounds[b]:bounds[b + 1]] - (k * NPC + b * BS)).astype(np.int64)
            lo = s < NHALF
            slo, rlo = s[lo], rel[lo]
            shi, rhi = s[~lo] - NHALF, rel[~lo]
            assert len(slo) <= CH * P and len(shi) <= CH * P, (len(slo), len(shi))
            lo_full = np.zeros(CH * P, np.int64)
            lo_full[:len(slo)] = slo
            hi_full = np.zeros(CH * P, np.int64)
            hi_full[:len(shi)] = shi
            idxlo[k, :, b * FW:(b + 1) * FW] = _wrap_idx(lo_full)
            idxhi[k, :, b * FW:(b + 1) * FW] = _wrap_idx(hi_full)
            for (rels, coff) in ((rlo, 0), (rhi, CH)):
                nr = len(rels)
                ch = np.arange(nr) // P + coff
                pp = np.arange(nr) % P
                dstrel[k, pp, b * CPB + ch] = rels

    # xT tiles per core: [BPC, 128 feat, BS nodes] fp16 (host layout transform)
    x_pad = np.zeros((NPAD, P), np.float32)
    x_pad[:N] = x
    xt = np.ascontiguousarray(
        x_pad.reshape(NCORES, BPC, BS, P).transpose(0, 1, 3, 2)).astype(np.float16)

    # dis replicated along features: [core][BS, BPC*H]
    dismat = np.ones((NCORES, P, BPC * H), np.float16)
    batf = np.full((NCORES, P, BPC), -1.0, np.float16)
    bat_pad = np.full(NPAD, -1.0, np.float32)
    bat_pad[:N] = batch.astype(np.float32)
    for k in range(NCORES):
        dblk = dis_pad[k * NPC:(k + 1) * NPC].reshape(BPC, BS).T  # [BS, BPC]
        dismat[k, :BS] = np.repeat(dblk[:, :, None], H, axis=2).reshape(
            BS, BPC * H).astype(np.float16)
        batf[k, :BS] = bat_pad[k * NPC:(k + 1) * NPC].reshape(BPC, BS).T

    iota_rep = np.tile(np.arange(BS, dtype=np.float16), (P, CPB))
    iota512 = np.tile(np.arange(G, dtype=np.float16), (P, 1))

    cnt = np.bincount(batch, minlength=G).astype(np.float32)
    recg = np.tile((1.0 / np.clip(cnt, 1.0, None))[None, :], (P, 1))

    common = {
        "W1f": np.asarray(W1, np.float16),
        "W2f": np.asarray(W2, np.float16),
        "Wlf": np.asarray(Wl, np.float32),
        "b1r": np.tile(np.asarray(b1, np.float16), (P, 1)),
        "b2r": np.tile(np.asarray(b2, np.float16), (P, 1)),
        "blc": np.asarray(bl, np.float32).reshape(C, 1),
        "iota_rep": np.ascontiguousarray(iota_rep),
        "iota512": np.ascontiguousarray(iota512),
        "recg": np.ascontiguousarray(recg, np.float32),
    }
    in_maps = []
    for k in range(NCORES):
        m = dict(common)
        m["xt"] = xt[k]
        m["idxlo"] = idxlo[k]
        m["idxhi"] = idxhi[k]
        m["dstrel"] = dstrel[k]
        m["dismat"] = dismat[k]
        m["batf"] = batf[k]
        in_maps.append(m)
    return in_maps


def _build():
    RG = [list(range(NCORES))]
    f32, f16 = mybir.dt.float32, mybir.dt.float16
    MUL, ADD, EQ = (mybir.AluOpType.mult, mybir.AluOpType.add,
                    mybir.AluOpType.is_equal)

    nc = bacc.Bacc("TRN2", target_bir_lowering=False, debug=False,
                   num_devices=NCORES, num_swdge_queues=NQ)

    def inp(name, shape, dt):
        return nc.dram_tensor(name, shape, dt, kind="ExternalInput").ap()

    xt = inp("xt", (BPC, P, BS), f16)
    idxlo = inp("idxlo", (P, BPC * FW), mybir.dt.int16)
    idxhi = inp("idxhi", (P, BPC * FW), mybir.dt.int16)
    dstrel = inp("dstrel", (P, BPC * CPB), f16)
    dismat = inp("dismat", (P, BPC * H), f16)
    batf = inp("batf", (P, BPC), f16)
    W1f = inp("W1f", (P, H), f16)
    W2f = inp("W2f", (P, H), f16)
    Wlf = inp("Wlf", (H, C), f32)
    b1r = inp("b1r", (P, H), f16)
    b2r = inp("b2r", (P, H), f16)
    blc = inp("blc", (C, 1), f32)
    iota_rep = inp("iota_rep", (P, CPB * BS), f16)
    iota512 = inp("iota512", (P, G), f16)
    recg = inp("recg", (P, G), f32)
    out = nc.dram_tensor("out", (G, C), f32, kind="ExternalOutput").ap()

    with tile.TileContext(nc) as tc:
        with tc.tile_pool(name="const", bufs=1) as cpool, \
             tc.tile_pool(name="dram", bufs=1, space="DRAM") as dpool, \
             tc.tile_pool(name="wtile", bufs=3) as wpool, \
             tc.tile_pool(name="gath", bufs=5) as gpool, \
             tc.tile_pool(name="oh", bufs=3) as ohpool, \
             tc.tile_pool(name="ep", bufs=3) as eppool, \
             tc.tile_pool(name="persist", bufs=1) as ppool:

            tab_shard = [dpool.tile([NPC, H], f16, tag=f"tsh{l}", name=f"tsh{l}")
                         for l in (1, 2)]
            tab_full = [dpool.tile([NPAD, H], f16, addr_space="Shared",
                                   tag=f"tfl{l}", name=f"tfl{l}") for l in (1, 2)]
            ar_in = dpool.tile([P, G], f32, tag="ar_in", name="ar_in")
            ar_out = dpool.tile([P, G], f32, addr_space="Shared",
                                tag="ar_out", name="ar_out")

            # ---- constants ---------------------------------------------
            W1_sb = cpool.tile([P, H], f16)
            nc.sync.dma_start(out=W1_sb[:], in_=W1f[:, :])
            dis_sb = cpool.tile([P, BPC * H], f16)
            nc.sync.dma_start(out=dis_sb[:], in_=dismat[:, :])
            b1_sb = cpool.tile([P, H], f16)
            nc.sync.dma_start(out=b1_sb[:], in_=b1r[:, :])
            W2_sb = cpool.tile([P, H], f16)
            nc.sync.dma_start(out=W2_sb[:], in_=W2f[:, :])
            b2_sb = cpool.tile([P, H], f16)
            nc.sync.dma_start(out=b2_sb[:], in_=b2r[:, :])
            idxlo_sb = cpool.tile([P, BPC * FW], mybir.dt.int16)
            nc.sync.dma_start(out=idxlo_sb[:], in_=idxlo[:, :])
            idxhi_sb = cpool.tile([P, BPC * FW], mybir.dt.int16)
            nc.sync.dma_start(out=idxhi_sb[:], in_=idxhi[:, :])
            dst_sb = cpool.tile([P, BPC * CPB], f16)
            nc.sync.dma_start(out=dst_sb[:], in_=dstrel[:, :])
            iota_sb = cpool.tile([P, CPB * BS], f16)
            nc.sync.dma_start(out=iota_sb[:], in_=iota_rep[:, :])
            iota512_sb = cpool.tile([P, G], f16)
            nc.sync.dma_start(out=iota512_sb[:], in_=iota512[:, :])
            bat_sb = cpool.tile([P, BPC], f16)
            nc.sync.dma_start(out=bat_sb[:], in_=batf[:, :])
            Wl_sb = cpool.tile([H, C], f32)
            nc.sync.dma_start(out=Wl_sb[:], in_=Wlf[:, :])
            bl_sb = cpool.tile([C, 1], f32)
            nc.sync.dma_start(out=bl_sb[:], in_=blc[:, :])
            recg_sb = cpool.tile([P, G], f32)
            nc.sync.dma_start(out=recg_sb[:], in_=recg[:, :])
            identf = cpool.tile([P, P], f32)
            make_identity(nc, identf[:])

            sf_sb = [ppool.tile([P, BPC * H], f16, tag=f"sf{l}", name=f"sf{l}")
                     for l in (1, 2)]

            def dis_blk(b):
                return dis_sb[:BS, b * H:(b + 1) * H]

            with tc.tile_pool(name="psAD", bufs=2, space="PSUM") as pspool, \
                 tc.tile_pool(name="psPool", bufs=1, space="PSUM") as plpool:
                ps_pool = plpool.tile([P, G], f32, tag="pool")

                # ---- layer-1 table + self rows -------------------------
                for t in range(BPC):
                    xf = wpool.tile([P, BS], f16, tag="xf")
                    nc.sync.dma_start(out=xf[:], in_=xt[t, :, :])
                    psm = pspool.tile([BS, H], f32, tag="psm")
                    nc.tensor.matmul(out=psm[:], lhsT=xf[:], rhs=W1_sb[:],
                                     start=True, stop=True)
                    tt = wpool.tile([BS, H], f16, tag="tt")
                    nc.vector.tensor_tensor(out=tt[:], in0=psm[:],
                                            in1=dis_blk(t), op=MUL)
                    nc.sync.dma_start(out=tab_shard[0][t * BS:(t + 1) * BS, :],
                                      in_=tt[:])
                    tsf = wpool.tile([BS, H], f16, tag="tsf")
                    nc.vector.tensor_tensor(out=tsf[:], in0=tt[:],
                                            in1=dis_blk(t), op=MUL)
                    nc.vector.tensor_tensor(
                        out=sf_sb[0][:BS, t * H:(t + 1) * H], in0=tsf[:],
                        in1=b1_sb[:BS, :], op=ADD)
                nc.gpsimd.collective_compute(
                    "AllGather", mybir.AluOpType.bypass, replica_groups=RG,
                    ins=[tab_shard[0][:, :]], outs=[tab_full[0][:, :]])

                # ---- scatter layers ------------------------------------
                def gather_block(l, b):
                    gt = gpool.tile([P, SPB], f16, tag="gt")
                    gt3 = gt[:].rearrange("p (c e) -> p c e", e=P)
                    nc.gpsimd.dma_gather(
                        out_ap=gt3[:, 0:CH, :], in_ap=tab_full[l][0:NHALF, :],
                        idxs_ap=idxlo_sb[:, b * FW:(b + 1) * FW],
                        num_idxs=CH * P, num_idxs_reg=CH * P, elem_size=H,
                        queue_num=(2 * b) % NQ)
                    nc.gpsimd.dma_gather(
                        out_ap=gt3[:, CH:CPB, :], in_ap=tab_full[l][NHALF:NPAD, :],
                        idxs_ap=idxhi_sb[:, b * FW:(b + 1) * FW],
                        num_idxs=CH * P, num_idxs_reg=CH * P, elem_size=H,
                        queue_num=(2 * b + 1) % NQ)
                    return gt

                def build_oh(b):
                    oh = ohpool.tile([P, CPB * BS], f16, tag="oh")
                    nc.vector.tensor_tensor(
                        out=oh[:].rearrange("p (c e) -> p c e", e=BS),
                        in0=dst_sb[:, b * CPB:(b + 1) * CPB][:, :, None]
                            .to_broadcast([P, CPB, BS]),
                        in1=iota_sb[:].rearrange("p (c e) -> p c e", e=BS),
                        op=EQ)
                    return oh

                def build_oh5(b):
                    oh5 = ohpool.tile([P, G], f16, tag="oh5")
                    nc.vector.tensor_tensor(
                        out=oh5[:],
                        in0=bat_sb[:, b:b + 1].to_broadcast([P, G]),
                        in1=iota512_sb[:], op=EQ)
                    return oh5

                def block_agg(gt, oh, b):
                    agg = pspool.tile([BS, H], f32, tag="agg")
                    for c in range(CPB):
                        nc.tensor.matmul(out=agg[:],
                                         lhsT=oh[:, c * BS:(c + 1) * BS],
                                         rhs=gt[:, c * P:(c + 1) * P],
                                         start=(c == 0), stop=(c == CPB - 1))
                    return agg

                def epilogue(agg, l, b, dt):
                    t1 = eppool.tile([BS, H], f16, tag="t1")
                    nc.vector.tensor_tensor(out=t1[:], in0=agg[:],
                                            in1=dis_blk(b), op=MUL)
                    u = eppool.tile([BS, H], f16, tag="u")
                    nc.vector.tensor_tensor(
                        out=u[:], in0=t1[:],
                        in1=sf_sb[l][:BS, b * H:(b + 1) * H], op=ADD)
                    ht = eppool.tile([BS, H], dt, tag="ht")
                    nc.scalar.activation(
                        out=ht[:], in_=u[:],
                        func=mybir.ActivationFunctionType.Relu)
                    return ht

                # layer 1 scatter, fused with layer-2 table build
                gts, ohs = {}, {}
                for j in range(LEAD_G):
                    gts[j] = gather_block(0, j)
                ohs[0] = build_oh(0)
                for b in range(BPC):
                    if b + LEAD_G < BPC:
                        gts[b + LEAD_G] = gather_block(0, b + LEAD_G)
                    if b + LEAD_O < BPC and b + LEAD_O not in ohs:
                        ohs[b + LEAD_O] = build_oh(b + LEAD_O)
                    agg = block_agg(gts.pop(b), ohs.pop(b), b)
                    h1t = epilogue(agg, 0, b, f32)
                    pst = pspool.tile([P, BS], f32, tag="pst")
                    nc.tensor.transpose(out=pst[:], in_=h1t[:, :],
                                        identity=identf[:BS, :BS])
                    hT = eppool.tile([P, BS], f16, tag="hT")
                    nc.scalar.activation(
                        out=hT[:], in_=pst[:],
                        func=mybir.ActivationFunctionType.Copy)
                    psm2 = pspool.tile([BS, H], f32, tag="psm")
                    nc.tensor.matmul(out=psm2[:], lhsT=hT[:], rhs=W2_sb[:],
                                     start=True, stop=True)
                    tt2 = eppool.tile([BS, H], f16, tag="tt2")
                    nc.vector.tensor_tensor(out=tt2[:], in0=psm2[:],
                                            in1=dis_blk(b), op=MUL)
                    nc.sync.dma_start(
                        out=tab_shard[1][b * BS:(b + 1) * BS, :], in_=tt2[:])
                    ts2 = eppool.tile([BS, H], f16, tag="ts2")
                    nc.vector.tensor_tensor(out=ts2[:], in0=tt2[:],
                                            in1=dis_blk(b), op=MUL)
                    nc.vector.tensor_tensor(
                        out=sf_sb[1][:BS, b * H:(b + 1) * H], in0=ts2[:],
                        in1=b2_sb[:BS, :], op=ADD)
                nc.gpsimd.collective_compute(
                    "AllGather", mybir.AluOpType.bypass, replica_groups=RG,
                    ins=[tab_shard[1][:, :]], outs=[tab_full[1][:, :]])

                # layer 2 scatter + pooling
                gts, ohs, oh5s = {}, {}, {}
                for j in range(LEAD_G):
                    gts[j] = gather_block(1, j)
                ohs[0] = build_oh(0)
                oh5s[0] = build_oh5(0)
                for b in range(BPC):
                    if b + LEAD_G < BPC:
                        gts[b + LEAD_G] = gather_block(1, b + LEAD_G)
                    if b + LEAD_O < BPC and b + LEAD_O not in ohs:
                        ohs[b + LEAD_O] = build_oh(b + LEAD_O)
                        oh5s[b + LEAD_O] = build_oh5(b + LEAD_O)
                    agg = block_agg(gts.pop(b), ohs.pop(b), b)
                    h2t = epilogue(agg, 1, b, f16)
                    oh5 = oh5s.pop(b)
                    nc.tensor.matmul(out=ps_pool[:], lhsT=h2t[:, :],
                                     rhs=oh5[:BS, :],
                                     start=(b == 0), stop=(b == BPC - 1))

                # ---- pooling tail --------------------------------------
                sums_sb = ppool.tile([P, G], f32, tag="sums")
                nc.vector.tensor_copy(out=sums_sb[:], in_=ps_pool[:])
                nc.sync.dma_start(out=ar_in[:, :], in_=sums_sb[:])
            nc.gpsimd.collective_compute(
                "AllReduce", mybir.AluOpType.add, replica_groups=RG,
                ins=[ar_in[:, :]], outs=[ar_out[:, :]])
            psE = tc.tile_pool(name="psE", bufs=1, space="PSUM")
            pspool = psE.__enter__()
            sums2 = ppool.tile([P, G], f32, tag="sums2")
            nc.sync.dma_start(out=sums2[:], in_=ar_out[:, :])
            means = ppool.tile([P, G], f32, tag="means")
            nc.vector.tensor_tensor(out=means[:], in0=sums2[:], in1=recg_sb[:],
                                    op=MUL)
            ps_out = pspool.tile([C, G], f32, tag="out")
            nc.tensor.matmul(out=ps_out[:], lhsT=Wl_sb[:], rhs=means[:],
                             start=True, stop=True)
            outT = ppool.tile([C, G], f32, tag="outT")
            nc.scalar.activation(out=outT[:], in_=ps_out[:],
                                 func=mybir.ActivationFunctionType.Identity,
                                 bias=bl_sb[:, 0:1])
            for g in range(G // P):
                ps_tr = pspool.tile([P, C], f32, tag="tr")
                nc.tensor.transpose(out=ps_tr[:],
                                    in_=outT[:, g * P:(g + 1) * P],
                                    identity=identf[:C, :C])
                ot = eppool.tile([P, C], f32, tag="ot")
                nc.vector.tensor_copy(out=ot[:], in_=ps_tr[:])
                nc.sync.dma_start(out=out[g * P:(g + 1) * P, :], in_=ot[:])
            psE.__exit__(None, None, None)

    nc.compile()
    return nc


def kernel(x, edge_index, batch, W1, b1, W2, b2, Wl, bl, _trace=False):
    in_maps = _prep(x, edge_index, batch, W1, b1, W2, b2, Wl, bl)
    if "nc" not in _cache:
        _cache["nc"] = _build()
    nc = _cache["nc"]
    res = bass_utils.run_bass_kernel_spmd(
        nc, in_maps, core_ids=list(range(NCORES)), trace=_trace)
    kernel.last_result = res
    return res.results[0]["out"].astype(np.float32)


# revision 46
# speedup vs baseline: 1.4960x; 1.0263x over previous
"""GCN (2-layer GCNConv + global mean pool + linear head) on 8 Trainium2 cores.

Strategy (follows the sharding hint):
  - Nodes sharded contiguously: core k owns nodes [6272k, 6272k+6272).
  - Edges sharded by destination; per dst-block (112 nodes) edge lists are
    built host-side (index-only preprocessing) and padded to uniform size.
  - GCNConv: out = dis * (sum_{e:dst=v} tab[src_e]) + sf with
    tab = (h @ W) * dis (fp16 table), sf = rec*(h@W) + b (self-loop+bias),
    dis = 1/sqrt(deg+1), rec = dis^2.
  - Neighbor rows are fetched with gpsimd.dma_gather (two <=32768-row table
    halves, int16 indices, 1024 idx/op), summed per dst block with a
    one-hot matmul on the PE (segment-sum). The block loop is software
    pipelined: gathers lead by 3 blocks, one-hot builds by 1.
  - Per-layer tables are shard-computed then AllGather'd (halo exchange);
    layer-2 table build is fused into the layer-1 scatter loop.
  - Pooling: one-hot (node->graph) matmul -> sums, AllReduce, mean (host
    precomputed 1/count) + linear head on-device.
All numeric compute runs on-device; the host only partitions/reorders
indices and pre-replicates per-node/per-graph normalization constants.
"""
import sys
import types

sys.path.insert(0, "/opt/trn_rl_repo")


def _install_ntff_hook():
    if "antenv.axon_hooks" in sys.modules:
        return
    mod = types.ModuleType("antenv.axon_hooks")
    mod._hook = None
    mod.set_axon_ntff_profile_hook = lambda h: setattr(mod, "_hook", h)
    mod.get_axon_ntff_profile_hook = lambda: mod._hook
    sys.modules["antenv.axon_hooks"] = mod
    sys.path.insert(0, "/root/.axon_site")
    try:
        from trn_agent_boot.trn_boot import _ntff_profile_via_ctypes
        mod.set_axon_ntff_profile_hook(
            _ntff_profile_via_ctypes("/opt/axon/libaxon_pjrt.so"))
    except Exception:
        pass


_install_ntff_hook()

import numpy as np
import concourse.bass as bass
import concourse.bacc as bacc
import concourse.mybir as mybir
import concourse.tile as tile
from concourse import bass_utils
from concourse.masks import make_identity

P = 128
N = 50000
E = 800000
H = 128
C = 10
G = 512
NCORES = 8
BS = 112                 # nodes per dst block
BPC = 56                 # dst blocks per core
NPC = BPC * BS           # nodes per core (6272)
NPAD = NCORES * NPC      # padded node count (50176)
NHALF = NPAD // 2        # table half rows (25088)
CH = 8                   # chunks per half per block (1024 idx = one gather)
CPB = 2 * CH             # chunks per block
SPB = CPB * P            # slots per block (2048)
FW = CH * P // 16        # idx columns per half per block (64)
NQ = 4
LEAD_G = 3               # gather lookahead (blocks)
LEAD_O = 1               # one-hot lookahead (blocks)

_cache = {}


def _wrap_idx(idx_flat):
    """dma_gather idx layout: position i -> [i%16, i//16], replicated x8."""
    n = idx_flat.shape[0]
    arr = np.ascontiguousarray(idx_flat.reshape(n // 16, 16).T).astype(np.int16)
    return np.tile(arr, (8, 1))


def _prep(x, edge_index, batch, W1, b1, W2, b2, Wl, bl):
    src = np.asarray(edge_index[0], np.int64)
    dst = np.asarray(edge_index[1], np.int64)
    batch = np.asarray(batch, np.int64)
    x = np.asarray(x, np.float32)

    deg = np.bincount(dst, minlength=N).astype(np.float32) + 1.0
    deg_pad = np.ones(NPAD, np.float32)
    deg_pad[:N] = deg
    dis_pad = 1.0 / np.sqrt(deg_pad)

    core_of = dst // NPC
    blk_of = (dst % NPC) // BS

    idxlo = np.zeros((NCORES, P, BPC * FW), np.int16)
    idxhi = np.zeros((NCORES, P, BPC * FW), np.int16)
    dstrel = np.full((NCORES, P, BPC * CPB), -1.0, np.float16)
    for k in range(NCORES):
        m = core_of == k
        sk, dk, bk = src[m], dst[m], blk_of[m]
        order = np.argsort(bk, kind="stable")
        sk, dk, bk = sk[order], dk[order], bk[order]
        bounds = np.searchsorted(bk, np.arange(BPC + 1))
        for b in range(BPC):
            s = sk[bounds[b]:bounds[b + 1]]
            rel = (dk[bounds[b]:bounds[b + 1]] - (k * NPC + b * BS)).astype(np.int64)
            lo = s < NHALF
            slo, rlo = s[lo], rel[lo]
            shi, rhi = s[~lo] - NHALF, rel[~lo]
            assert len(slo) <= CH * P and len(shi) <= CH * P, (len(slo), len(shi))
            lo_full = np.zeros(CH * P, np.int64)
            lo_full[:len(slo)] = slo
            hi_full = np.zeros(CH * P, np.int64)
            hi_full[:len(shi)] = shi
            idxlo[k, :, b * FW:(b + 1) * FW] = _wrap_idx(lo_full)
            idxhi[k, :, b * FW:(b + 1) * FW] = _wrap_idx(hi_full)
            for (rels, coff) in ((rlo, 0), (rhi, CH)):
                nr = len(rels)
                ch = np.arange(nr) // P + coff
                pp = np.arange(nr) % P
                dstrel[k, pp, b * CPB + ch] = rels

    # xT tiles per core: [BPC, 128 feat, BS nodes] fp16 (host layout transform)
    x_pad = np.zeros((NPAD, P), np.float32)
    x_pad[:N] = x
    xt = np.ascontiguousarray(
        x_pad.reshape(NCORES, BPC, BS, P).transpose(0, 1, 3, 2)).astype(np.float16)

    # dis replicated along features: [core][BS, BPC*H]
    dismat = np.ones((NCORES, P, BPC * H), np.float16)
    batf = np.full((NCORES, P, BPC), -1.0, np.float16)
    bat_pad = np.full(NPAD, -1.0, np.float32)
    bat_pad[:N] = batch.astype(np.float32)
    for k in range(NCORES):
        dblk = dis_pad[k * NPC:(k + 1) * NPC].reshape(BPC, BS).T  # [BS, BPC]
        dismat[k, :BS] = np.repeat(dblk[:, :, None], H, axis=2).reshape(
            BS, BPC * H).astype(np.float16)
        batf[k, :BS] = bat_pad[k * NPC:(k + 1) * NPC].reshape(BPC, BS).T

    iota_rep = np.tile(np.arange(BS, dtype=np.float16), (P, CPB))
    iota512 = np.tile(np.arange(G, dtype=np.float16), (P, 1))

    cnt = np.bincount(batch, minlength=G).astype(np.float32)
    recg = np.tile((1.0 / np.clip(cnt, 1.0, None))[None, :], (P, 1))

    common = {
        "W1f": np.asarray(W1, np.float16),
        "W2f": np.asarray(W2, np.float16),
        "Wlf": np.asarray(Wl, np.float32),
        "b1r": np.tile(np.asarray(b1, np.float16), (P, 1)),
        "b2r": np.tile(np.asarray(b2, np.float16), (P, 1)),
        "blc": np.asarray(bl, np.float32).reshape(C, 1),
        "iota_rep": np.ascontiguousarray(iota_rep),
        "iota512": np.ascontiguousarray(iota512),
        "recg": np.ascontiguousarray(recg, np.float32),
    }
    in_maps = []
    for k in range(NCORES):
        m = dict(common)
        m["xt"] = xt[k]
        m["idxlo"] = idxlo[k]
        m["idxhi"] = idxhi[k]
        m["dstrel"] = dstrel[k]
        m["dismat"] = dismat[k]
        m["batf"] = batf[k]
        in_maps.append(m)
    return in_maps


def _build():
    RG = [list(range(NCORES))]
    f32, f16 = mybir.dt.float32, mybir.dt.float16
    MUL, ADD, EQ = (mybir.AluOpType.mult, mybir.AluOpType.add,
                    mybir.AluOpType.is_equal)

    nc = bacc.Bacc("TRN2", target_bir_lowering=False, debug=False,
                   num_devices=NCORES, num_swdge_queues=NQ)

    def inp(name, shape, dt):
        return nc.dram_tensor(name, shape, dt, kind="ExternalInput").ap()

    xt = inp("xt", (BPC, P, BS), f16)
    idxlo = inp("idxlo", (P, BPC * FW), mybir.dt.int16)
    idxhi = inp("idxhi", (P, BPC * FW), mybir.dt.int16)
    dstrel = inp("dstrel", (P, BPC * CPB), f16)
    dismat = inp("dismat", (P, BPC * H), f16)
    batf = inp("batf", (P, BPC), f16)
    W1f = inp("W1f", (P, H), f16)
    W2f = inp("W2f", (P, H), f16)
    Wlf = inp("Wlf", (H, C), f32)
    b1r = inp("b1r", (P, H), f16)
    b2r = inp("b2r", (P, H), f16)
    blc = inp("blc", (C, 1), f32)
    iota_rep = inp("iota_rep", (P, CPB * BS), f16)
    iota512 = inp("iota512", (P, G), f16)
    recg = inp("recg", (P, G), f32)
    out = nc.dram_tensor("out", (G, C), f32, kind="ExternalOutput").ap()

    with tile.TileContext(nc) as tc:
        with tc.tile_pool(name="const", bufs=1) as cpool, \
             tc.tile_pool(name="dram", bufs=1, space="DRAM") as dpool, \
             tc.tile_pool(name="wtile", bufs=3) as wpool, \
             tc.tile_pool(name="gath", bufs=5) as gpool, \
             tc.tile_pool(name="oh", bufs=3) as ohpool, \
             tc.tile_pool(name="ep", bufs=3) as eppool, \
             tc.tile_pool(name="persist", bufs=1) as ppool:

            tab_shard = [dpool.tile([NPC, H], f16, tag=f"tsh{l}", name=f"tsh{l}")
                         for l in (1, 2)]
            tab_full = [dpool.tile([NPAD, H], f16, addr_space="Shared",
                                   tag=f"tfl{l}", name=f"tfl{l}") for l in (1, 2)]
            ar_in = dpool.tile([P, G], f32, tag="ar_in", name="ar_in")
            ar_out = dpool.tile([P, G], f32, addr_space="Shared",
                                tag="ar_out", name="ar_out")

            # ---- constants ---------------------------------------------
            W1_sb = cpool.tile([P, H], f16)
            nc.sync.dma_start(out=W1_sb[:], in_=W1f[:, :])
            dis_sb = cpool.tile([P, BPC * H], f16)
            nc.sync.dma_start(out=dis_sb[:], in_=dismat[:, :])
            b1_sb = cpool.tile([P, H], f16)
            nc.sync.dma_start(out=b1_sb[:], in_=b1r[:, :])
            W2_sb = cpool.tile([P, H], f16)
            nc.sync.dma_start(out=W2_sb[:], in_=W2f[:, :])
            b2_sb = cpool.tile([P, H], f16)
            nc.sync.dma_start(out=b2_sb[:], in_=b2r[:, :])
            idxlo_sb = cpool.tile([P, BPC * FW], mybir.dt.int16)
            nc.sync.dma_start(out=idxlo_sb[:], in_=idxlo[:, :])
            idxhi_sb = cpool.tile([P, BPC * FW], mybir.dt.int16)
            nc.sync.dma_start(out=idxhi_sb[:], in_=idxhi[:, :])
            dst_sb = cpool.tile([P, BPC * CPB], f16)
            nc.sync.dma_start(out=dst_sb[:], in_=dstrel[:, :])
            iota_sb = cpool.tile([P, CPB * BS], f16)
            nc.sync.dma_start(out=iota_sb[:], in_=iota_rep[:, :])
            iota512_sb = cpool.tile([P, G], f16)
            nc.sync.dma_start(out=iota512_sb[:], in_=iota512[:, :])
            bat_sb = cpool.tile([P, BPC], f16)
            nc.sync.dma_start(out=bat_sb[:], in_=batf[:, :])
            Wl_sb = cpool.tile([H, C], f32)
            nc.sync.dma_start(out=Wl_sb[:], in_=Wlf[:, :])
            bl_sb = cpool.tile([C, 1], f32)
            nc.sync.dma_start(out=bl_sb[:], in_=blc[:, :])
            recg_sb = cpool.tile([P, G], f32)
            nc.sync.dma_start(out=recg_sb[:], in_=recg[:, :])
            identf = cpool.tile([P, P], f32)
            make_identity(nc, identf[:])

            sf_sb = [ppool.tile([P, BPC * H], f16, tag=f"sf{l}", name=f"sf{l}")
                     for l in (1, 2)]

            def dis_blk(b):
                return dis_sb[:BS, b * H:(b + 1) * H]

            with tc.tile_pool(name="psAD", bufs=2, space="PSUM") as pspool, \
                 tc.tile_pool(name="psPool", bufs=1, space="PSUM") as plpool:
                ps_pool = plpool.tile([P, G], f32, tag="pool")

                # ---- layer-1 table + self rows -------------------------
                for t in range(BPC):
                    xf = wpool.tile([P, BS], f16, tag="xf")
                    nc.sync.dma_start(out=xf[:], in_=xt[t, :, :])
                    psm = pspool.tile([BS, H], f32, tag="psm")
                    nc.tensor.matmul(out=psm[:], lhsT=xf[:], rhs=W1_sb[:],
                                     start=True, stop=True)
                    tt = wpool.tile([BS, H], f16, tag="tt")
                    nc.vector.tensor_tensor(out=tt[:], in0=psm[:],
                                            in1=dis_blk(t), op=MUL)
                    nc.sync.dma_start(out=tab_shard[0][t * BS:(t + 1) * BS, :],
                                      in_=tt[:])
                    tsf = wpool.tile([BS, H], f16, tag="tsf")
                    nc.vector.tensor_tensor(out=tsf[:], in0=tt[:],
                                            in1=dis_blk(t), op=MUL)
                    nc.vector.tensor_tensor(
                        out=sf_sb[0][:BS, t * H:(t + 1) * H], in0=tsf[:],
                        in1=b1_sb[:BS, :], op=ADD)
                nc.gpsimd.collective_compute(
                    "AllGather", mybir.AluOpType.bypass, replica_groups=RG,
                    ins=[tab_shard[0][:, :]], outs=[tab_full[0][:, :]])

                # ---- scatter layers ------------------------------------
                def gather_block(l, b):
                    gt = gpool.tile([P, SPB], f16, tag="gt")
                    gt3 = gt[:].rearrange("p (c e) -> p c e", e=P)
                    nc.gpsimd.dma_gather(
                        out_ap=gt3[:, 0:CH, :], in_ap=tab_full[l][0:NHALF, :],
                        idxs_ap=idxlo_sb[:, b * FW:(b + 1) * FW],
                        num_idxs=CH * P, num_idxs_reg=CH * P, elem_size=H,
                        queue_num=(2 * b) % NQ)
                    nc.gpsimd.dma_gather(
                        out_ap=gt3[:, CH:CPB, :], in_ap=tab_full[l][NHALF:NPAD, :],
                        idxs_ap=idxhi_sb[:, b * FW:(b + 1) * FW],
                        num_idxs=CH * P, num_idxs_reg=CH * P, elem_size=H,
                        queue_num=(2 * b + 1) % NQ)
                    return gt

                def build_oh(b):
                    oh = ohpool.tile([P, CPB * BS], f16, tag="oh")
                    nc.vector.tensor_tensor(
                        out=oh[:].rearrange("p (c e) -> p c e", e=BS),
                        in0=dst_sb[:, b * CPB:(b + 1) * CPB][:, :, None]
                            .to_broadcast([P, CPB, BS]),
                        in1=iota_sb[:].rearrange("p (c e) -> p c e", e=BS),
                        op=EQ)
                    return oh

                def build_oh5(b):
                    oh5 = ohpool.tile([P, G], f16, tag="oh5")
                    nc.vector.tensor_tensor(
                        out=oh5[:],
                        in0=bat_sb[:, b:b + 1].to_broadcast([P, G]),
                        in1=iota512_sb[:], op=EQ)
                    return oh5

                def block_agg(gt, oh, b):
                    agg = pspool.tile([BS, H], f32, tag="agg")
                    for c in range(CPB):
                        nc.tensor.matmul(out=agg[:],
                                         lhsT=oh[:, c * BS:(c + 1) * BS],
                                         rhs=gt[:, c * P:(c + 1) * P],
                                         start=(c == 0), stop=(c == CPB - 1))
                    return agg

                def epilogue(agg, l, b, dt):
                    t1 = eppool.tile([BS, H], f16, tag="t1")
                    nc.vector.tensor_tensor(out=t1[:], in0=agg[:],
                                            in1=dis_blk(b), op=MUL)
                    u = eppool.tile([BS, H], f16, tag="u")
                    nc.vector.tensor_tensor(
                        out=u[:], in0=t1[:],
                        in1=sf_sb[l][:BS, b * H:(b + 1) * H], op=ADD)
                    ht = eppool.tile([BS, H], dt, tag="ht")
                    nc.scalar.activation(
                        out=ht[:], in_=u[:],
                        func=mybir.ActivationFunctionType.Relu)
                    return ht

                # layer 1 scatter, fused with layer-2 table build
                gts, ohs = {}, {}
                for j in range(LEAD_G):
                    gts[j] = gather_block(0, j)
                ohs[0] = build_oh(0)
                for b in range(BPC):
                    if b + LEAD_G < BPC:
                        gts[b + LEAD_G] = gather_block(0, b + LEAD_G)
                    if b + LEAD_O < BPC and b + LEAD_O not in ohs:
                        ohs[b + LEAD_O] = build_oh(b + LEAD_O)
                    agg = block_agg(gts.pop(b), ohs.pop(b), b)
                    h1t = epilogue(agg, 0, b, f32)
                    pst = pspool.tile([P, BS], f32, tag="pst")
                    nc.tensor.transpose(out=pst[:], in_=h1t[:, :],
                                        identity=identf[:BS, :BS])
                    hT = eppool.tile([P, BS], f16, tag="hT")
                    nc.scalar.activation(
                        out=hT[:], in_=pst[:],
                        func=mybir.ActivationFunctionType.Copy)
                    psm2 = pspool.tile([BS, H], f32, tag="psm")
                    nc.tensor.matmul(out=psm2[:], lhsT=hT[:], rhs=W2_sb[:],
                                     start=True, stop=True)
                    tt2 = eppool.tile([BS, H], f16, tag="tt2")
                    nc.vector.tensor_tensor(out=tt2[:], in0=psm2[:],
                                            in1=dis_blk(b), op=MUL)
                    nc.sync.dma_start(
                        out=tab_shard[1][b * BS:(b + 1) * BS, :], in_=tt2[:])
                    ts2 = eppool.tile([BS, H], f16, tag="ts2")
                    nc.vector.tensor_tensor(out=ts2[:], in0=tt2[:],
                                            in1=dis_blk(b), op=MUL)
                    nc.vector.tensor_tensor(
                        out=sf_sb[1][:BS, b * H:(b + 1) * H], in0=ts2[:],
                        in1=b2_sb[:BS, :], op=ADD)
                nc.gpsimd.collective_compute(
                    "AllGather", mybir.AluOpType.bypass, replica_groups=RG,
                    ins=[tab_shard[1][:, :]], outs=[tab_full[1][:, :]])

                # layer 2 scatter + pooling
                gts, ohs, oh5s = {}, {}, {}
                for j in range(LEAD_G):
                    gts[j] = gather_block(1, j)
                ohs[0] = build_oh(0)
                oh5s[0] = build_oh5(0)
                for b in range(BPC):
                    if b + LEAD_G < BPC:
                        gts[b + LEAD_G] = gather_block(1, b + LEAD_G)
                    if b + LEAD_O < BPC and b + LEAD_O not in ohs:
                        ohs[b + LEAD_O] = build_oh(b + LEAD_O)
                        oh5s[b + LEAD_O] = build_oh5(b + LEAD_O)
                    agg = block_agg(gts.pop(b), ohs.pop(b), b)
                    h2t = epilogue(agg, 1, b, f16)
                    oh5 = oh5s.pop(b)
                    nc.tensor.matmul(out=ps_pool[:], lhsT=h2t[:, :],
                                     rhs=oh5[:BS, :],
                                     start=(b == 0), stop=(b == BPC - 1))

                # ---- pooling tail --------------------------------------
                sums_sb = ppool.tile([P, G], f32, tag="sums")
                nc.vector.tensor_copy(out=sums_sb[:], in_=ps_pool[:])
                nc.sync.dma_start(out=ar_in[:, :], in_=sums_sb[:])
            nc.gpsimd.collective_compute(
                "AllReduce", mybir.AluOpType.add, replica_groups=RG,
                ins=[ar_in[:, :]], outs=[ar_out[:, :]])
            psE = tc.tile_pool(name="psE", bufs=1, space="PSUM")
            pspool = psE.__enter__()
            sums2 = ppool.tile([P, G], f32, tag="sums2")
            nc.sync.dma_start(out=sums2[:], in_=ar_out[:, :])
            means = ppool.tile([P, G], f32, tag="means")
            nc.vector.tensor_tensor(out=means[:], in0=sums2[:], in1=recg_sb[:],
                                    op=MUL)
            ps_out = pspool.tile([C, G], f32, tag="out")
            nc.tensor.matmul(out=ps_out[:], lhsT=Wl_sb[:], rhs=means[:],
                             start=True, stop=True)
            outT = ppool.tile([C, G], f32, tag="outT")
            nc.scalar.activation(out=outT[:], in_=ps_out[:],
                                 func=mybir.ActivationFunctionType.Identity,
                                 bias=bl_sb[:, 0:1])
            for g in range(G // P):
                ps_tr = pspool.tile([P, C], f32, tag="tr")
                nc.tensor.transpose(out=ps_tr[:],
                                    in_=outT[:, g * P:(g + 1) * P],
                                    identity=identf[:C, :C])
                ot = eppool.tile([P, C], f32, tag="ot")
                nc.vector.tensor_copy(out=ot[:], in_=ps_tr[:])
                nc.sync.dma_start(out=out[g * P:(g + 1) * P, :], in_=ot[:])
            psE.__exit__(None, None, None)

    nc.compile()
    return nc


def kernel(x, edge_index, batch, W1, b1, W2, b2, Wl, bl, _trace=False):
    in_maps = _prep(x, edge_index, batch, W1, b1, W2, b2, Wl, bl)
    if "nc" not in _cache:
        _cache["nc"] = _build()
    nc = _cache["nc"]
    res = bass_utils.run_bass_kernel_spmd(
        nc, in_maps, core_ids=list(range(NCORES)), trace=_trace)
    kernel.last_result = res
    return res.results[0]["out"].astype(np.float32)
